# Initial kernel scaffold
#
"""Trainium2 Bass kernel: 4-branch GNN message passing (scatter-mean propagation).

Strategy (8 NeuronCores, SPMD):
  - Nodes are sharded across the 8 cores (4096 dest nodes per core); the
    small weight matrices are replicated.
  - Branch i of the reference needs i scatter-mean propagations.  The six
    propagations are restructured into 3 stacked passes over column blocks:
        pass 1: [h1|h2|h3] (384 cols), pass 2: [h2'|h3'] (256), pass 3: [h3''].
  - Before each pass the per-core rows are AllGather'd so every core holds the
    full [32768, C] operand; each core then computes its own 4096 dest rows:
    a batched dma_gather pulls the source rows of its edges (sorted by dest,
    padded to uniform 128-edge tiles per 128-dest window), a one-hot matrix
    built on-device (iota + is_equal) turns the segment-sum into TF32
    (float32r) matmuls accumulated in PSUM, and the flush applies 1/deg.
  - The per-branch MLPs, gates, concat and output projection are local,
    computed feature-major so biases are per-partition scalars; gates are
    folded into Wo on the host.  The final output is returned feature-major
    per core and transposed/concatenated on the host.
"""

import os

import numpy as np

N = 32768
E = 524288
IN_F = 256
OUT_F = 128
NB = 4
NCORES = 8
LOCAL = N // NCORES          # 4096
P = 128
NWIN = LOCAL // P            # 32 dest windows per core
C1, C2, C3 = 3 * OUT_F, 2 * OUT_F, OUT_F
NODE_CHUNK = 512

LAST_EXEC_NS = None
_PROG_CACHE = {}


def _install_ntff_hook():
    """Make run_bass_kernel_spmd(trace=True) work when antenv.axon_hooks is absent."""
    import sys
    import types

    try:
        import antenv.axon_hooks  # noqa: F401
        return
    except ImportError:
        pass
    try:
        from trn_agent_boot.trn_boot import _ntff_profile_via_ctypes
    except ImportError:
        return
    hook = _ntff_profile_via_ctypes("/opt/axon/libaxon_pjrt.so")
    mod = types.ModuleType("antenv.axon_hooks")
    mod.get_axon_ntff_profile_hook = lambda: hook
    mod.set_axon_ntff_profile_hook = lambda h: None
    sys.modules["antenv.axon_hooks"] = mod


def _build_program(T):
    import concourse.bacc as bacc
    import concourse.mybir as mybir
    import concourse.tile as tile
    from concourse.library_config import mlp as mlp_lib

    f32 = mybir.dt.float32
    f32r = mybir.dt.float32r
    i16 = mybir.dt.int16

    nc = bacc.Bacc("TRN2", target_bir_lowering=False)

    # ---- per-core external inputs -------------------------------------------------
    xt = nc.dram_tensor("xt", [2, P, LOCAL], f32r, kind="ExternalInput")
    gidx = nc.dram_tensor("gidx", [P, NWIN * T * 8], i16, kind="ExternalInput")
    destl = nc.dram_tensor("destl", [P, NWIN * T], f32, kind="ExternalInput")
    invd = nc.dram_tensor("invd", [P, NWIN], f32, kind="ExternalInput")
    iota_t = nc.dram_tensor("iota_t", [P, P], f32, kind="ExternalInput")
    iota_c = nc.dram_tensor("iota_c", [P, 1], f32, kind="ExternalInput")
    wb = nc.dram_tensor("wb", [2, P, NB * OUT_F], f32r, kind="ExternalInput")
    bbr = nc.dram_tensor("bbr", [1, NB * OUT_F], f32r, kind="ExternalInput")
    w1s = nc.dram_tensor("w1s", [P, NB * P], f32r, kind="ExternalInput")
    b1s = nc.dram_tensor("b1s", [P, NB], f32, kind="ExternalInput")
    w2s = nc.dram_tensor("w2s", [P, NB * P], f32r, kind="ExternalInput")
    b2s = nc.dram_tensor("b2s", [P, NB], f32, kind="ExternalInput")
    wos = nc.dram_tensor("wos", [P, NB * P], f32r, kind="ExternalInput")
    boc = nc.dram_tensor("boc", [P, 1], f32, kind="ExternalInput")

    outT = nc.dram_tensor("outT", [P, LOCAL], f32, kind="ExternalOutput")

    with tile.TileContext(nc) as tc:
        nc.gpsimd.load_library(mlp_lib)

        # ---- internal DRAM (tile-tracked) ----------------------------------------
        hb1 = tc.tile([LOCAL, C1], f32r, space="DRAM", name="hb1")
        hg1 = tc.tile([N, C1], f32r, space="DRAM", addr_space="Shared", name="hg1")
        hb2 = tc.tile([LOCAL, C2], f32r, space="DRAM", name="hb2")
        hg2 = tc.tile([N, C2], f32r, space="DRAM", addr_space="Shared", name="hg2")
        hb3 = tc.tile([LOCAL, C3], f32r, space="DRAM", name="hb3")
        hg3 = tc.tile([N, C3], f32r, space="DRAM", addr_space="Shared", name="hg3")
        h0d = tc.tile([LOCAL, P], f32r, space="DRAM", name="h0d")
        ob1 = tc.tile([LOCAL, P], f32r, space="DRAM", name="ob1")
        ob2 = tc.tile([LOCAL, P], f32r, space="DRAM", name="ob2")
        ob3 = tc.tile([LOCAL, P], f32r, space="DRAM", name="ob3")

        with (
            tc.tile_pool(name="const", bufs=1) as cpool,
            tc.tile_pool(name="work", bufs=2) as work,
            tc.tile_pool(name="gath", bufs=3) as gpool,
            tc.tile_pool(name="psA", bufs=2, space="PSUM") as psA,
            tc.tile_pool(name="psB", bufs=1, space="PSUM") as psB,
        ):
            # ---- resident constants ----------------------------------------------
            gidx_sb = cpool.tile([P, NWIN * T * 8], i16)
            nc.sync.dma_start(gidx_sb[:], gidx[:])
            destl_sb = cpool.tile([P, NWIN * T], f32)
            nc.sync.dma_start(destl_sb[:], destl[:])
            invd_sb = cpool.tile([P, NWIN], f32)
            nc.sync.dma_start(invd_sb[:], invd[:])
            iota_sb = cpool.tile([P, P], f32)
            nc.sync.dma_start(iota_sb[:], iota_t[:])
            iotac_sb = cpool.tile([P, 1], f32)
            nc.sync.dma_start(iotac_sb[:], iota_c[:])
            wb_sb = cpool.tile([P, 2 * NB * OUT_F], f32r)
            nc.sync.dma_start(wb_sb[:, : NB * OUT_F], wb[0])
            nc.sync.dma_start(wb_sb[:, NB * OUT_F :], wb[1])
            bb_sb = cpool.tile([1, NB * OUT_F], f32r)
            nc.sync.dma_start(bb_sb[:], bbr[:])
            w1_sb = cpool.tile([P, NB * P], f32r)
            nc.sync.dma_start(w1_sb[:], w1s[:])
            w2_sb = cpool.tile([P, NB * P], f32r)
            nc.sync.dma_start(w2_sb[:], w2s[:])
            wo_sb = cpool.tile([P, NB * P], f32r)
            nc.sync.dma_start(wo_sb[:], wos[:])
            b1_sb = cpool.tile([P, NB], f32)
            nc.sync.dma_start(b1_sb[:], b1s[:])
            b2_sb = cpool.tile([P, NB], f32)
            nc.sync.dma_start(b2_sb[:], b2s[:])
            bo_sb = cpool.tile([P, 1], f32)
            nc.sync.dma_start(bo_sb[:], boc[:])

            # identity (for PE transpose) and an all-ones row (for bias matmuls)
            ident = cpool.tile([P, P], f32r)
            nc.vector.tensor_tensor(
                out=ident[:],
                in0=iotac_sb[:].to_broadcast([P, P]),
                in1=iota_sb[:],
                op=mybir.AluOpType.is_equal,
            )
            ones_sb = cpool.tile([1, P], f32r)
            nc.vector.tensor_tensor(
                out=ones_sb[:],
                in0=iota_sb[0:1, :],
                in1=iota_sb[0:1, :],
                op=mybir.AluOpType.is_equal,
            )

            # ---- phase 0: h = x @ Wb + bb for all 4 branches ---------------------
            for nt in range(NWIN):
                sl = slice(nt * P, (nt + 1) * P)
                xt0 = work.tile([P, P], f32r, tag="xt0")
                nc.sync.dma_start(xt0[:], xt[0, :, sl])
                xt1 = work.tile([P, P], f32r, tag="xt1")
                nc.sync.dma_start(xt1[:], xt[1, :, sl])
                ps0 = psA.tile([P, NB * OUT_F], f32, tag="ps0")
                nc.tensor.matmul(
                    ps0[:], lhsT=xt0[:], rhs=wb_sb[:, : NB * OUT_F],
                    start=True, stop=False,
                )
                nc.tensor.matmul(
                    ps0[:], lhsT=xt1[:], rhs=wb_sb[:, NB * OUT_F :],
                    start=False, stop=False,
                )
                nc.tensor.matmul(
                    ps0[:], lhsT=ones_sb[0:1, :], rhs=bb_sb[0:1, :],
                    start=False, stop=True,
                )
                hsb = work.tile([P, NB * OUT_F], f32r, tag="hsb")
                nc.vector.tensor_copy(hsb[:], ps0[:])
                nc.sync.dma_start(h0d[sl, :], hsb[:, :OUT_F])
                nc.sync.dma_start(hb1[sl, :], hsb[:, OUT_F:])

            # ---- propagation passes ----------------------------------------------
            rg = [list(range(NCORES))]

            def allgather(src, dst):
                nc.gpsimd.collective_compute(
                    "AllGather",
                    mybir.AluOpType.bypass,
                    replica_groups=rg,
                    ins=[src[:]],
                    outs=[dst[:]],
                )

            def prop_pass(src_hg, C, dsts, pname):
                # dsts: list of (col_lo, col_hi, dram_dst)
                for w in range(NWIN):
                    gt = gpool.tile([P, T, C], f32r, tag="gath")
                    nc.gpsimd.dma_gather(
                        gt[:],
                        src_hg[:],
                        gidx_sb[:, w * T * 8 : (w + 1) * T * 8],
                        T * P,
                        T * P,
                        C,
                    )
                    oh = work.tile([P, T, P], f32r, tag="oh")
                    nc.vector.tensor_tensor(
                        out=oh[:],
                        in0=destl_sb[:, w * T : (w + 1) * T, None].to_broadcast(
                            [P, T, P]
                        ),
                        in1=iota_sb[:, None, :].to_broadcast([P, T, P]),
                        op=mybir.AluOpType.is_equal,
                    )
                    psw = psA.tile([P, C], f32, tag="psw")
                    for t in range(T):
                        nc.tensor.matmul(
                            psw[:],
                            lhsT=oh[:, t, :],
                            rhs=gt[:, t, :],
                            start=(t == 0),
                            stop=(t == T - 1),
                        )
                    sc = work.tile([P, C], f32r, tag="sc")
                    nc.vector.tensor_scalar_mul(sc[:], psw[:], invd_sb[:, w : w + 1])
                    rsl = slice(w * P, (w + 1) * P)
                    for lo, hi, dram in dsts:
                        nc.sync.dma_start(dram[rsl, :], sc[:, lo:hi])

            allgather(hb1, hg1)
            prop_pass(hg1, C1, [(0, P, ob1), (P, C1, hb2)], "p1")
            allgather(hb2, hg2)
            prop_pass(hg2, C2, [(0, P, ob2), (P, C2, hb3)], "p2")
            allgather(hb3, hg3)
            prop_pass(hg3, C3, [(0, P, ob3)], "p3")

            # ---- MLP + gated concat + output projection (feature-major) ----------
            srcs = [h0d, ob1, ob2, ob3]
            for ch in range(LOCAL // NODE_CHUNK):
                outp = psB.tile([P, NODE_CHUNK], f32, tag="outp")
                for i in range(NB):
                    hT = work.tile([P, NODE_CHUNK], f32r, tag="hT")
                    for j in range(NODE_CHUNK // P):
                        r0 = ch * NODE_CHUNK + j * P
                        ht = work.tile([P, P], f32r, tag="ht")
                        nc.sync.dma_start(ht[:], srcs[i][r0 : r0 + P, :])
                        pst = psB.tile([P, P], f32r, tag="pst")
                        nc.tensor.transpose(out=pst[:], in_=ht[:], identity=ident[:])
                        nc.vector.tensor_copy(hT[:, j * P : (j + 1) * P], pst[:])
                    zp = psB.tile([P, NODE_CHUNK], f32, tag="zp")
                    nc.tensor.matmul(
                        zp[:], lhsT=w1_sb[:, i * P : (i + 1) * P], rhs=hT[:],
                        start=True, stop=True,
                    )
                    zr = work.tile([P, NODE_CHUNK], f32r, tag="zr")
                    nc.scalar.activation(
                        zr[:], zp[:], mybir.ActivationFunctionType.Relu,
                        bias=b1_sb[:, i : i + 1],
                    )
                    yp = psB.tile([P, NODE_CHUNK], f32, tag="yp")
                    nc.tensor.matmul(
                        yp[:], lhsT=w2_sb[:, i * P : (i + 1) * P], rhs=zr[:],
                        start=True, stop=True,
                    )
                    yb = work.tile([P, NODE_CHUNK], f32r, tag="yb")
                    nc.vector.tensor_scalar_add(yb[:], yp[:], b2_sb[:, i : i + 1])
                    nc.tensor.matmul(
                        outp[:], lhsT=wo_sb[:, i * P : (i + 1) * P], rhs=yb[:],
                        start=(i == 0), stop=(i == NB - 1),
                    )
                fin = work.tile([P, NODE_CHUNK], f32, tag="fin")
                nc.vector.tensor_scalar_add(fin[:], outp[:], bo_sb[:, 0:1])
                nc.sync.dma_start(
                    outT[:, ch * NODE_CHUNK : (ch + 1) * NODE_CHUNK], fin[:]
                )

    nc.compile()
    return nc


def _preprocess(inputs):
    x = np.asarray(inputs["x"], dtype=np.float32)
    ei = np.asarray(inputs["edge_index"])
    row = ei[0].astype(np.int64)
    col = ei[1].astype(np.int64)

    deg = np.bincount(col, minlength=N).astype(np.float32)
    inv_deg = (1.0 / np.maximum(deg, 1.0)).astype(np.float32)

    order = np.argsort(col, kind="stable")
    rs = row[order]
    cs = col[order]
    wb_bounds = np.searchsorted(cs, np.arange(0, N + P, P))
    counts = np.diff(wb_bounds)
    T = int(np.ceil(counts.max() / P))

    iota_row = np.tile(np.arange(P, dtype=np.float32)[None, :], (P, 1))
    iota_col = np.arange(P, dtype=np.float32)[:, None]

    Wb = np.asarray(inputs["Wb"], np.float32)
    bb = np.asarray(inputs["bb"], np.float32)
    W1 = np.asarray(inputs["W1"], np.float32)
    b1 = np.asarray(inputs["b1"], np.float32)
    W2 = np.asarray(inputs["W2"], np.float32)
    b2 = np.asarray(inputs["b2"], np.float32)
    Wo = np.asarray(inputs["Wo"], np.float32)
    bo = np.asarray(inputs["bo"], np.float32)
    bg = np.asarray(inputs["branch_gates"], np.float32)
    temp = np.asarray(inputs["temperature"], np.float32)

    g = bg / temp
    g = np.exp(g - g.max())
    gates = (g / g.sum()).astype(np.float32)

    wb_cat = np.concatenate([Wb[i] for i in range(NB)], axis=1)  # [256, 512]
    shared = {
        "wb": np.ascontiguousarray(wb_cat.reshape(2, P, NB * OUT_F)),
        "bbr": np.concatenate([bb[i] for i in range(NB)])[None, :].copy(),
        "w1s": np.concatenate([W1[i] for i in range(NB)], axis=1).copy(),
        "b1s": np.stack([b1[i] for i in range(NB)], axis=1).copy(),
        "w2s": np.concatenate([W2[i] for i in range(NB)], axis=1).copy(),
        "b2s": np.stack([b2[i] for i in range(NB)], axis=1).copy(),
        "wos": np.concatenate(
            [gates[i] * Wo[i * P : (i + 1) * P, :] for i in range(NB)], axis=1
        ).copy(),
        "boc": bo[:, None].copy(),
        "iota_t": iota_row,
        "iota_c": iota_col,
    }

    in_maps = []
    for c in range(NCORES):
        gidx_rows = np.zeros((NWIN, T * P), np.int16)
        dl = np.full((NWIN, T * P), -1.0, np.float32)
        for w in range(NWIN):
            gw = c * NWIN + w
            e0, e1 = wb_bounds[gw], wb_bounds[gw + 1]
            cnt = e1 - e0
            gidx_rows[w, :cnt] = rs[e0:e1].astype(np.int16)
            dl[w, :cnt] = (cs[e0:e1] - gw * P).astype(np.float32)
        gi = gidx_rows.reshape(NWIN, T * 8, 16).transpose(0, 2, 1)  # [NWIN,16,T*8]
        gi = np.tile(gi, (1, 8, 1))  # [NWIN,128,T*8]
        gidx_sb = np.ascontiguousarray(gi.transpose(1, 0, 2).reshape(P, NWIN * T * 8))
        destl_sb = np.ascontiguousarray(
            dl.reshape(NWIN, T, P).transpose(2, 0, 1).reshape(P, NWIN * T)
        )
        invd_sb = np.ascontiguousarray(
            inv_deg[c * LOCAL : (c + 1) * LOCAL].reshape(NWIN, P).T
        )
        xt_c = np.ascontiguousarray(
            x[c * LOCAL : (c + 1) * LOCAL].T.reshape(2, P, LOCAL)
        )
        m = dict(shared)
        m.update(gidx=gidx_sb, destl=destl_sb, invd=invd_sb, xt=xt_c)
        in_maps.append(m)
    return T, in_maps


def kernel(**inputs) -> np.ndarray:
    global LAST_EXEC_NS
    from concourse.bass_utils import run_bass_kernel_spmd

    T, in_maps = _preprocess(inputs)
    if T not in _PROG_CACHE:
        _PROG_CACHE[T] = _build_program(T)
    nc = _PROG_CACHE[T]

    trace = bool(os.environ.get("KERNEL_TRACE"))
    if trace:
        _install_ntff_hook()
    res = run_bass_kernel_spmd(nc, in_maps, list(range(NCORES)), trace=trace)
    LAST_EXEC_NS = res.exec_time_ns

    out = np.empty((N, OUT_F), np.float32)
    for c in range(NCORES):
        out[c * LOCAL : (c + 1) * LOCAL, :] = res.results[c]["outT"].T
    return out


# revision 5
# speedup vs baseline: 7.4086x; 7.4086x over previous
"""Trainium2 Bass kernel: 4-branch GNN message passing (scatter-mean propagation).

Strategy (8 NeuronCores, SPMD):
  - Nodes are sharded across the 8 cores (4096 dest nodes per core); the
    small weight matrices are replicated.
  - Branch i of the reference needs i scatter-mean propagations.  The six
    propagations are restructured into 3 stacked passes over column blocks:
        pass 1: [h1|h2|h3] (384 cols), pass 2: [h2'|h3'] (256), pass 3: [h3''].
  - Before each pass the per-core rows are AllGather'd so every core holds the
    full [32768, C] operand; each core then computes its own 4096 dest rows:
    a batched dma_gather pulls the source rows of its edges (sorted by dest,
    padded to uniform 128-edge tiles per 128-dest window), a one-hot matrix
    built on-device (iota + is_equal) turns the segment-sum into TF32
    (float32r) matmuls accumulated in PSUM, and the flush applies 1/deg.
  - The per-branch MLPs, gates, concat and output projection are local,
    computed feature-major so biases are per-partition scalars; gates are
    folded into Wo on the host.  The final output is returned feature-major
    per core and transposed/concatenated on the host.
"""

import os

import numpy as np

N = 32768
E = 524288
IN_F = 256
OUT_F = 128
NB = 4
NCORES = 8
LOCAL = N // NCORES          # 4096
P = 128
NWIN = LOCAL // P            # 32 dest windows per core
C1, C2, C3 = 3 * OUT_F, 2 * OUT_F, OUT_F
NODE_CHUNK = 512

LAST_EXEC_NS = None
_PROG_CACHE = {}


def _install_ntff_hook():
    """Make run_bass_kernel_spmd(trace=True) work when antenv.axon_hooks is absent."""
    import sys
    import types

    try:
        import antenv.axon_hooks  # noqa: F401
        return
    except ImportError:
        pass
    try:
        from trn_agent_boot.trn_boot import _ntff_profile_via_ctypes
    except ImportError:
        return
    hook = _ntff_profile_via_ctypes("/opt/axon/libaxon_pjrt.so")
    mod = types.ModuleType("antenv.axon_hooks")
    mod.get_axon_ntff_profile_hook = lambda: hook
    mod.set_axon_ntff_profile_hook = lambda h: None
    sys.modules["antenv.axon_hooks"] = mod


def _build_program(T):
    import concourse.bacc as bacc
    import concourse.mybir as mybir
    import concourse.tile as tile
    from concourse.library_config import mlp as mlp_lib

    f32 = mybir.dt.float32
    f32r = mybir.dt.float32r
    i16 = mybir.dt.int16

    nc = bacc.Bacc("TRN2", target_bir_lowering=False)

    # ---- per-core external inputs -------------------------------------------------
    xt = nc.dram_tensor("xt", [2, P, LOCAL], f32r, kind="ExternalInput")
    gidx = nc.dram_tensor("gidx", [P, NWIN * T * 8], i16, kind="ExternalInput")
    destl = nc.dram_tensor("destl", [P, NWIN * T], f32, kind="ExternalInput")
    invd = nc.dram_tensor("invd", [P, NWIN], f32, kind="ExternalInput")
    iota_t = nc.dram_tensor("iota_t", [P, P], f32, kind="ExternalInput")
    iota_c = nc.dram_tensor("iota_c", [P, 1], f32, kind="ExternalInput")
    wb = nc.dram_tensor("wb", [2, P, NB * OUT_F], f32r, kind="ExternalInput")
    bbr = nc.dram_tensor("bbr", [1, NB * OUT_F], f32r, kind="ExternalInput")
    w1s = nc.dram_tensor("w1s", [P, NB * P], f32r, kind="ExternalInput")
    b1s = nc.dram_tensor("b1s", [P, NB], f32, kind="ExternalInput")
    w2s = nc.dram_tensor("w2s", [P, NB * P], f32r, kind="ExternalInput")
    b2s = nc.dram_tensor("b2s", [P, NB], f32, kind="ExternalInput")
    wos = nc.dram_tensor("wos", [P, NB * P], f32r, kind="ExternalInput")
    boc = nc.dram_tensor("boc", [P, 1], f32, kind="ExternalInput")

    outT = nc.dram_tensor("outT", [P, LOCAL], f32, kind="ExternalOutput")

    with tile.TileContext(nc) as tc:
        nc.gpsimd.load_library(mlp_lib)

        # ---- internal DRAM buffers -----------------------------------------------
        hb1 = nc.dram_tensor("hb1", [LOCAL, C1], f32r)
        hg1 = nc.dram_tensor("hg1", [N, C1], f32r, addr_space="Shared")
        hb2 = nc.dram_tensor("hb2", [LOCAL, C2], f32r)
        hg2 = nc.dram_tensor("hg2", [N, C2], f32r, addr_space="Shared")
        hb3 = nc.dram_tensor("hb3", [LOCAL, C3], f32r)
        hg3 = nc.dram_tensor("hg3", [N, C3], f32r, addr_space="Shared")
        h0d = nc.dram_tensor("h0d", [LOCAL, P], f32r)
        ob1 = nc.dram_tensor("ob1", [LOCAL, P], f32r)
        ob2 = nc.dram_tensor("ob2", [LOCAL, P], f32r)
        ob3 = nc.dram_tensor("ob3", [LOCAL, P], f32r)

        with (
            tc.tile_pool(name="const", bufs=1) as cpool,
            tc.tile_pool(name="work", bufs=2) as work,
            tc.tile_pool(name="gath", bufs=3) as gpool,
            tc.tile_pool(name="psA", bufs=2, space="PSUM") as psA,
            tc.tile_pool(name="psB", bufs=1, space="PSUM") as psB,
        ):
            # ---- resident constants ----------------------------------------------
            gidx_sb = cpool.tile([P, NWIN * T * 8], i16)
            nc.sync.dma_start(gidx_sb[:], gidx[:])
            destl_sb = cpool.tile([P, NWIN * T], f32)
            nc.sync.dma_start(destl_sb[:], destl[:])
            invd_sb = cpool.tile([P, NWIN], f32)
            nc.sync.dma_start(invd_sb[:], invd[:])
            iota_sb = cpool.tile([P, P], f32)
            nc.sync.dma_start(iota_sb[:], iota_t[:])
            iotac_sb = cpool.tile([P, 1], f32)
            nc.sync.dma_start(iotac_sb[:], iota_c[:])
            wb_sb = cpool.tile([P, 2 * NB * OUT_F], f32r)
            nc.sync.dma_start(wb_sb[:, : NB * OUT_F], wb[0])
            nc.sync.dma_start(wb_sb[:, NB * OUT_F :], wb[1])
            bb_sb = cpool.tile([1, NB * OUT_F], f32r)
            nc.sync.dma_start(bb_sb[:], bbr[:])
            w1_sb = cpool.tile([P, NB * P], f32r)
            nc.sync.dma_start(w1_sb[:], w1s[:])
            w2_sb = cpool.tile([P, NB * P], f32r)
            nc.sync.dma_start(w2_sb[:], w2s[:])
            wo_sb = cpool.tile([P, NB * P], f32r)
            nc.sync.dma_start(wo_sb[:], wos[:])
            b1_sb = cpool.tile([P, NB], f32)
            nc.sync.dma_start(b1_sb[:], b1s[:])
            b2_sb = cpool.tile([P, NB], f32)
            nc.sync.dma_start(b2_sb[:], b2s[:])
            bo_sb = cpool.tile([P, 1], f32)
            nc.sync.dma_start(bo_sb[:], boc[:])

            # identity (for PE transpose) and an all-ones row (for bias matmuls)
            ident = cpool.tile([P, P], f32r)
            nc.vector.tensor_tensor(
                out=ident[:],
                in0=iotac_sb[:].to_broadcast([P, P]),
                in1=iota_sb[:],
                op=mybir.AluOpType.is_equal,
            )
            ones_sb = cpool.tile([1, P], f32r)
            nc.vector.tensor_tensor(
                out=ones_sb[:],
                in0=iota_sb[0:1, :],
                in1=iota_sb[0:1, :],
                op=mybir.AluOpType.is_equal,
            )

            # ---- phase 0: h = x @ Wb + bb for all 4 branches ---------------------
            for nt in range(NWIN):
                sl = slice(nt * P, (nt + 1) * P)
                xt0 = work.tile([P, P], f32r, tag="xt0")
                nc.sync.dma_start(xt0[:], xt[0, :, sl])
                xt1 = work.tile([P, P], f32r, tag="xt1")
                nc.sync.dma_start(xt1[:], xt[1, :, sl])
                ps0 = psA.tile([P, NB * OUT_F], f32, tag="ps0")
                nc.tensor.matmul(
                    ps0[:], lhsT=xt0[:], rhs=wb_sb[:, : NB * OUT_F],
                    start=True, stop=False,
                )
                nc.tensor.matmul(
                    ps0[:], lhsT=xt1[:], rhs=wb_sb[:, NB * OUT_F :],
                    start=False, stop=False,
                )
                nc.tensor.matmul(
                    ps0[:], lhsT=ones_sb[0:1, :], rhs=bb_sb[0:1, :],
                    start=False, stop=True,
                )
                hsb = work.tile([P, NB * OUT_F], f32r, tag="hsb")
                nc.vector.tensor_copy(hsb[:], ps0[:])
                nc.sync.dma_start(h0d[sl, :], hsb[:, :OUT_F])
                nc.sync.dma_start(hb1[sl, :], hsb[:, OUT_F:])

            # ---- propagation passes ----------------------------------------------
            rg = [list(range(NCORES))]

            def allgather(src, dst):
                nc.gpsimd.collective_compute(
                    "AllGather",
                    mybir.AluOpType.bypass,
                    replica_groups=rg,
                    ins=[src[:]],
                    outs=[dst[:]],
                )

            def prop_pass(src_hg, C, dsts, pname):
                # dsts: list of (col_lo, col_hi, dram_dst)
                for w in range(NWIN):
                    gt = gpool.tile([P, T, C], f32r, tag="gath")
                    nc.gpsimd.dma_gather(
                        gt[:],
                        src_hg[:],
                        gidx_sb[:, w * T * 8 : (w + 1) * T * 8],
                        T * P,
                        T * P,
                        C,
                    )
                    oh = work.tile([P, T, P], f32r, tag="oh")
                    nc.vector.tensor_tensor(
                        out=oh[:],
                        in0=destl_sb[:, w * T : (w + 1) * T, None].to_broadcast(
                            [P, T, P]
                        ),
                        in1=iota_sb[:, None, :].to_broadcast([P, T, P]),
                        op=mybir.AluOpType.is_equal,
                    )
                    psw = psA.tile([P, C], f32, tag="psw")
                    for t in range(T):
                        nc.tensor.matmul(
                            psw[:],
                            lhsT=oh[:, t, :],
                            rhs=gt[:, t, :],
                            start=(t == 0),
                            stop=(t == T - 1),
                        )
                    sc = work.tile([P, C], f32r, tag="sc")
                    nc.vector.tensor_scalar_mul(sc[:], psw[:], invd_sb[:, w : w + 1])
                    rsl = slice(w * P, (w + 1) * P)
                    for lo, hi, dram in dsts:
                        nc.sync.dma_start(dram[rsl, :], sc[:, lo:hi])

            phases = int(os.environ.get("KERNEL_PHASES", "3"))
            p1mode = os.environ.get("KERNEL_P1MODE", "full")
            if phases >= 1:
                allgather(hb1, hg1)
                if p1mode == "full":
                    prop_pass(hg1, C1, [(0, P, ob1), (P, C1, hb2)], "p1")
                elif p1mode == "copy":
                    # debug: plain DMA read from hg1 instead of dma_gather
                    for w in range(2):
                        dbg = work.tile([P, C1], f32r, tag="dbg")
                        nc.sync.dma_start(dbg[:], hg1[w * P : (w + 1) * P, :])
                        nc.sync.dma_start(ob1[w * P : (w + 1) * P, :], dbg[:])
            if phases >= 2:
                allgather(hb2, hg2)
                prop_pass(hg2, C2, [(0, P, ob2), (P, C2, hb3)], "p2")
            if phases >= 3:
                allgather(hb3, hg3)
                prop_pass(hg3, C3, [(0, P, ob3)], "p3")

            # ---- MLP + gated concat + output projection (feature-major) ----------
            srcs = [h0d, ob1, ob2, ob3]
            mlp_on = os.environ.get("KERNEL_MLP", "1") != "0"
            for ch in range(LOCAL // NODE_CHUNK if mlp_on else 0):
                outp = psB.tile([P, NODE_CHUNK], f32, tag="outp")
                for i in range(NB):
                    hT = work.tile([P, NODE_CHUNK], f32r, tag="hT")
                    for j in range(NODE_CHUNK // P):
                        r0 = ch * NODE_CHUNK + j * P
                        ht = work.tile([P, P], f32r, tag="ht")
                        nc.sync.dma_start(ht[:], srcs[i][r0 : r0 + P, :])
                        pst = psB.tile([P, P], f32r, tag="pst")
                        nc.tensor.transpose(out=pst[:], in_=ht[:], identity=ident[:])
                        nc.vector.tensor_copy(hT[:, j * P : (j + 1) * P], pst[:])
                    zp = psB.tile([P, NODE_CHUNK], f32, tag="zp")
                    nc.tensor.matmul(
                        zp[:], lhsT=w1_sb[:, i * P : (i + 1) * P], rhs=hT[:],
                        start=True, stop=True,
                    )
                    zr = work.tile([P, NODE_CHUNK], f32r, tag="zr")
                    nc.scalar.activation(
                        zr[:], zp[:], mybir.ActivationFunctionType.Relu,
                        bias=b1_sb[:, i : i + 1],
                    )
                    yp = psB.tile([P, NODE_CHUNK], f32, tag="yp")
                    nc.tensor.matmul(
                        yp[:], lhsT=w2_sb[:, i * P : (i + 1) * P], rhs=zr[:],
                        start=True, stop=True,
                    )
                    yb = work.tile([P, NODE_CHUNK], f32r, tag="yb")
                    nc.vector.tensor_scalar_add(yb[:], yp[:], b2_sb[:, i : i + 1])
                    nc.tensor.matmul(
                        outp[:], lhsT=wo_sb[:, i * P : (i + 1) * P], rhs=yb[:],
                        start=(i == 0), stop=(i == NB - 1),
                    )
                fin = work.tile([P, NODE_CHUNK], f32, tag="fin")
                nc.vector.tensor_scalar_add(fin[:], outp[:], bo_sb[:, 0:1])
                nc.sync.dma_start(
                    outT[:, ch * NODE_CHUNK : (ch + 1) * NODE_CHUNK], fin[:]
                )

    nc.compile()
    return nc


def _preprocess(inputs):
    x = np.asarray(inputs["x"], dtype=np.float32)
    ei = np.asarray(inputs["edge_index"])
    row = ei[0].astype(np.int64)
    col = ei[1].astype(np.int64)

    deg = np.bincount(col, minlength=N).astype(np.float32)
    inv_deg = (1.0 / np.maximum(deg, 1.0)).astype(np.float32)

    order = np.argsort(col, kind="stable")
    rs = row[order]
    cs = col[order]
    wb_bounds = np.searchsorted(cs, np.arange(0, N + P, P))
    counts = np.diff(wb_bounds)
    T = int(np.ceil(counts.max() / P))

    iota_row = np.tile(np.arange(P, dtype=np.float32)[None, :], (P, 1))
    iota_col = np.arange(P, dtype=np.float32)[:, None]

    Wb = np.asarray(inputs["Wb"], np.float32)
    bb = np.asarray(inputs["bb"], np.float32)
    W1 = np.asarray(inputs["W1"], np.float32)
    b1 = np.asarray(inputs["b1"], np.float32)
    W2 = np.asarray(inputs["W2"], np.float32)
    b2 = np.asarray(inputs["b2"], np.float32)
    Wo = np.asarray(inputs["Wo"], np.float32)
    bo = np.asarray(inputs["bo"], np.float32)
    bg = np.asarray(inputs["branch_gates"], np.float32)
    temp = np.asarray(inputs["temperature"], np.float32)

    g = bg / temp
    g = np.exp(g - g.max())
    gates = (g / g.sum()).astype(np.float32)

    wb_cat = np.concatenate([Wb[i] for i in range(NB)], axis=1)  # [256, 512]
    shared = {
        "wb": np.ascontiguousarray(wb_cat.reshape(2, P, NB * OUT_F)),
        "bbr": np.concatenate([bb[i] for i in range(NB)])[None, :].copy(),
        "w1s": np.concatenate([W1[i] for i in range(NB)], axis=1).copy(),
        "b1s": np.stack([b1[i] for i in range(NB)], axis=1).copy(),
        "w2s": np.concatenate([W2[i] for i in range(NB)], axis=1).copy(),
        "b2s": np.stack([b2[i] for i in range(NB)], axis=1).copy(),
        "wos": np.concatenate(
            [gates[i] * Wo[i * P : (i + 1) * P, :] for i in range(NB)], axis=1
        ).copy(),
        "boc": bo[:, None].copy(),
        "iota_t": iota_row,
        "iota_c": iota_col,
    }

    in_maps = []
    for c in range(NCORES):
        gidx_rows = np.zeros((NWIN, T * P), np.int16)
        dl = np.full((NWIN, T * P), -1.0, np.float32)
        for w in range(NWIN):
            gw = c * NWIN + w
            e0, e1 = wb_bounds[gw], wb_bounds[gw + 1]
            cnt = e1 - e0
            gidx_rows[w, :cnt] = rs[e0:e1].astype(np.int16)
            dl[w, :cnt] = (cs[e0:e1] - gw * P).astype(np.float32)
        gi = gidx_rows.reshape(NWIN, T * 8, 16).transpose(0, 2, 1)  # [NWIN,16,T*8]
        gi = np.tile(gi, (1, 8, 1))  # [NWIN,128,T*8]
        gidx_sb = np.ascontiguousarray(gi.transpose(1, 0, 2).reshape(P, NWIN * T * 8))
        destl_sb = np.ascontiguousarray(
            dl.reshape(NWIN, T, P).transpose(2, 0, 1).reshape(P, NWIN * T)
        )
        invd_sb = np.ascontiguousarray(
            inv_deg[c * LOCAL : (c + 1) * LOCAL].reshape(NWIN, P).T
        )
        xt_c = np.ascontiguousarray(
            x[c * LOCAL : (c + 1) * LOCAL].T.reshape(2, P, LOCAL)
        )
        m = dict(shared)
        m.update(gidx=gidx_sb, destl=destl_sb, invd=invd_sb, xt=xt_c)
        in_maps.append(m)
    return T, in_maps


def kernel(**inputs) -> np.ndarray:
    global LAST_EXEC_NS
    from concourse.bass_utils import run_bass_kernel_spmd

    T, in_maps = _preprocess(inputs)
    if T not in _PROG_CACHE:
        _PROG_CACHE[T] = _build_program(T)
    nc = _PROG_CACHE[T]

    trace = bool(os.environ.get("KERNEL_TRACE"))
    if trace:
        _install_ntff_hook()
    res = run_bass_kernel_spmd(nc, in_maps, list(range(NCORES)), trace=trace)
    LAST_EXEC_NS = res.exec_time_ns

    out = np.empty((N, OUT_F), np.float32)
    for c in range(NCORES):
        out[c * LOCAL : (c + 1) * LOCAL, :] = res.results[c]["outT"].T
    return out
